# revision 13
# baseline (speedup 1.0000x reference)
"""ConditionalConv Trainium2 kernel.

Reference computation (B=32, CIN=COUT=32, K=3, H=W=128, COND_DIM=256):
    h = relu(cond @ W1.T + b1)          # [B, 4608]
    w = h @ W2.T + b2                   # [B, 9216] -> per-sample conv weights
    out[b] = conv2d(x[b], w[b])         # 3x3, stride 1, pad 1, per-sample

Distribution over 8 NeuronCores:
  Launch A (hyper-MLP): tensor-shard W2 along its 9216-output dim; every
    core computes h for all B samples (replicated, tiny) and its 1152-wide
    slice of w for all samples.  Host concatenates slices + adds b2.
  Launch B (conv): data-parallel over batch, 4 samples per core.  The four
    samples' [CIN, COUT] weight blocks per 3x3 tap are packed block-diagonally
    into a [128, 128] stationary operand, so each matmul contracts over
    4*CIN = 128 partitions and produces 4*COUT = 128 output channels at once.
    The conv is 9 accumulated matmuls per 512-pixel output tile against a
    host-pre-padded [128, 130, 130] image resident in SBUF.

Perf notes vs the first working version:
  - All hot streams trigger from the two HWDGE queues (sync, scalar);
    gpsimd SWDGE descriptor-gen (~670ns/DMA + expensive dge drains) was
    throttling the W2 stream to ~300 GB/s.
  - W2 stream pieces are bigger (fewer, larger transfers).
  - mlp1's tiny (32-row) matmuls are interleaved *between* mlp2's 384-row
    matmuls so every LDWEIGHTS hides behind a long matmul.
  - Conv stores are batched 2 tiles (8 rows) per DMA, alternating the two
    HWDGE trigger queues.

Precision: W2 is sent as fp8 e3m4 (4 mantissa bits), host-scaled by 64 into
e3m4's normal range and descaled in the PSUM->SBUF copy.  The stationary hT
stays fp16 (mixed-dtype matmul).  Everything else fp16 with fp32 PSUM."""

import numpy as np
import ml_dtypes

import concourse.bass as bass
import concourse.mybir as mybir
import concourse.tile as tile
from concourse import bacc
from concourse.bass_utils import run_bass_kernel_spmd

NCORES = 8
B, CIN, COUT, KK = 32, 32, 32, 3
H = W = 128
COND = 256
NPAR = CIN * COUT * KK * KK   # 9216
HID = NPAR // 2               # 4608
PSH = NPAR // NCORES          # 1152 params per core = 4 couts
BSH = B // NCORES             # 4 samples per core
HCH = HID // 128              # 36 hidden chunks of 128
W2SCALE = 64.0                # host-side scale into e3m4 normal range
HP, WP = H + 2, W + 2         # padded image

F32 = mybir.dt.float32
F16 = mybir.dt.float16
F8E3 = mybir.dt.float8e3

_cache = {}


def _build_mlp():
    nc = bacc.Bacc(
        "TRN2", target_bir_lowering=False, debug=False, enable_asserts=True,
        num_devices=NCORES,
    )
    condT = nc.dram_tensor("condT", [128, 2, B], F16, kind="ExternalInput").ap()
    # chunk-major: [part, hid-chunk, ci, 128] so any contiguous run of
    # chunks streams as one contiguous per-partition block
    w1t = nc.dram_tensor("W1T", [128, HCH, 2, 128], F16,
                         kind="ExternalInput").ap()
    b1 = nc.dram_tensor("b1", [128, HCH], F32, kind="ExternalInput").ap()
    w2t = nc.dram_tensor("W2T", [128, HCH, PSH], F8E3, kind="ExternalInput").ap()
    wout = nc.dram_tensor("wsh", [B, PSH], F16, kind="ExternalOutput").ap()

    # variable-size stream pieces: small leading pieces so the per-chunk
    # MLP pipeline starts as soon as possible, bigger ones for efficiency.
    # Few pieces total: each DMA trigger costs ~650ns of HWDGE engine time.
    W1P = [4, 16, 16]                              # w1t pieces (hid chunks)
    W2P = [2, 3, 4, 6, 7, 7, 7]                    # W2T groups (hid chunks)
    assert sum(W1P) == HCH and sum(W2P) == HCH
    W1OF = [sum(W1P[:i]) for i in range(len(W1P) + 1)]
    W2OF = [sum(W2P[:i]) for i in range(len(W2P) + 1)]
    with tile.TileContext(nc) as tc:
        with (
            tc.tile_pool(name="consts", bufs=1) as consts,
            tc.tile_pool(name="w2pool", bufs=1) as w2pool,
            tc.tile_pool(name="hpsum", bufs=4, space="PSUM") as hpsum,
            tc.tile_pool(name="wpsum", bufs=1, space="PSUM") as wpsum,
        ):
            condT_sb = consts.tile([128, 2, B], F16, tag="condT")
            w1t_sb = consts.tile([128, HCH, 2, 128], F16, tag="w1t")
            b1_sb = consts.tile([128, HCH], F32, tag="b1")
            hT_sb = consts.tile([128, HCH, B], F16, tag="hT")
            w_sb = consts.tile([B, PSH], F16, tag="w")
            warm = consts.tile([128, 512], F16, tag="warm")

            # Stream schedule: interleave w1t pieces and W2T groups in the
            # order the per-chunk MLP pipeline consumes them, alternating
            # across the two HWDGE trigger queues.
            # tiny constants first so chunk 0 can start the moment its
            # w1/w2 pieces land
            nc.sync.dma_start(condT_sb[:], condT)
            nc.scalar.dma_start(b1_sb[:], b1)
            # then interleave stream pieces in consumption order, w2 first
            w2gs = [
                w2pool.tile([128, W2P[g], PSH], F8E3, tag=f"w2g{g}",
                            name=f"w2g{g}")
                for g in range(len(W2P))
            ]

            def piece_of(offsets, hj):
                for c in range(len(offsets) - 1):
                    if offsets[c] <= hj < offsets[c + 1]:
                        return c
                raise AssertionError

            sched = []   # ("w1", piece) / ("w2", group) in consumption order
            seen_c, seen_g = set(), set()
            for hj in range(HCH):
                c, g = piece_of(W1OF, hj), piece_of(W2OF, hj)
                if g not in seen_g:
                    seen_g.add(g)
                    sched.append(("w2", g))
                if c not in seen_c:
                    seen_c.add(c)
                    sched.append(("w1", c))
            trig = [nc.sync, nc.scalar]
            for k, (kind, idx) in enumerate(sched):
                q = trig[k % 2]
                if kind == "w1":
                    sl = slice(W1OF[idx], W1OF[idx + 1])
                    q.dma_start(w1t_sb[:, sl], w1t[:, sl])
                else:
                    q.dma_start(
                        w2gs[idx][:],
                        w2t[:, W2OF[idx]:W2OF[idx + 1], :],
                    )

            # PE p-state warmup while the first stream pieces land: this
            # time is stream-wait anyway, and ~3us of continuous matmul
            # gets the PE to full clock before the real chunks start
            nc.vector.memset(warm[:], 0.0)
            pww = wpsum.tile([B, 512], F32, tag="pww")
            for wi in range(3):
                nc.tensor.matmul(
                    pww[:], warm[:, :B], warm[:, :], start=True, stop=True,
                )

            # ---- fused per-chunk pipeline over the 36 hidden chunks ----
            # mlp1(hj): ph = W1T-chunk.T @ condT        (PE, psum)
            # relu(hj): hT[:,hj,:] = relu(ph + b1)      (DVE)
            # mlp2(hj): w-psum += hT[:,hj,:].T @ W2T    (PE, 3 psum slices)
            # mlp1(hj+1)'s two 32-row matmuls are emitted BETWEEN mlp2(hj)'s
            # three 384-row matmuls so each LDWEIGHTS hides behind a long
            # matmul and the DVE relu latency is covered.
            pw0 = wpsum.tile([B, 384], F32, tag="pw0")
            pw1 = wpsum.tile([B, 384], F32, tag="pw1")
            pw2 = wpsum.tile([B, 384], F32, tag="pw2")
            pws = [(pw0, 0, 384), (pw1, 384, 384), (pw2, 768, 384)]

            def mlp1_mm(hj, ci, ph):
                nc.tensor.matmul(
                    ph[:],
                    w1t_sb[:, hj, ci, :],
                    condT_sb[:, ci, :],
                    start=(ci == 0),
                    stop=(ci == 1),
                )

            def relu(hj, ph):
                # bias (per-partition) + relu + round-to-f16 in one DVE op
                nc.vector.tensor_scalar(
                    hT_sb[:, hj, :], ph[:], b1_sb[:, hj:hj + 1], 0.0,
                    mybir.AluOpType.add, mybir.AluOpType.max,
                )

            def mlp2_mm(hj, k):
                g = piece_of(W2OF, hj)
                pt, p0, pn = pws[k]
                nc.tensor.matmul(
                    pt[:, :pn],
                    hT_sb[:, hj, :],
                    w2gs[g][:, hj - W2OF[g], p0:p0 + pn],
                    start=(hj == 0),
                    stop=(hj == HCH - 1),
                )

            ph_prev = hpsum.tile([128, B], F32, tag="ph")
            mlp1_mm(0, 0, ph_prev)
            mlp1_mm(0, 1, ph_prev)
            relu(0, ph_prev)
            for hj in range(HCH):
                ph = None
                if hj + 1 < HCH:
                    ph = hpsum.tile([128, B], F32, tag="ph")
                mlp2_mm(hj, 0)
                if ph is not None:
                    mlp1_mm(hj + 1, 0, ph)
                mlp2_mm(hj, 1)
                if ph is not None:
                    mlp1_mm(hj + 1, 1, ph)
                    relu(hj + 1, ph)
                mlp2_mm(hj, 2)
            for k, (pt, p0, pn) in enumerate(pws):
                # descale by 64 during the PSUM->SBUF copy
                if k == 1:
                    nc.scalar.activation(
                        w_sb[:, p0:p0 + pn], pt[:, :pn],
                        mybir.ActivationFunctionType.Copy,
                        scale=1.0 / W2SCALE,
                    )
                else:
                    nc.vector.tensor_scalar(
                        w_sb[:, p0:p0 + pn],
                        pt[:, :pn],
                        1.0 / W2SCALE,
                        0.0,
                        mybir.AluOpType.mult,
                        mybir.AluOpType.bypass,
                    )
            nc.sync.dma_start(wout, w_sb[:])
    nc.compile()
    return nc


def _build_conv():
    nc = bacc.Bacc(
        "TRN2", target_bir_lowering=False, debug=False, enable_asserts=True,
        num_devices=NCORES,
    )
    # x arrives host-pre-padded: [BSH, CIN, 130, 130] with zero borders
    xs = nc.dram_tensor("xs", [BSH, CIN, HP, WP], F16, kind="ExternalInput").ap()
    wst = nc.dram_tensor("wst", [128, KK * KK, 128], F16,
                         kind="ExternalInput").ap()
    ys = nc.dram_tensor("ys", [BSH, COUT, H, W], F16, kind="ExternalOutput").ap()

    xv = xs.rearrange("s c h w -> (s c) h w")   # [128, 130, 130]
    yv = ys.rearrange("s c h w -> (s c) h w")

    with tile.TileContext(nc) as tc:
        with (
            tc.tile_pool(name="sb", bufs=1) as sb,
            tc.tile_pool(name="outp", bufs=3) as outp,
            tc.tile_pool(name="cpsum", bufs=8, space="PSUM") as cpsum,
        ):
            xp = sb.tile([128, HP, WP], F16, tag="xp")
            wst_sb = sb.tile([128, KK * KK, 128], F16, tag="wst")
            nc.scalar.dma_start(wst_sb[:], wst)
            # row chunks; each partition reads contiguous bytes per chunk.
            # finer chunks at the head let the first matmuls start sooner
            bounds = [0, 6, 14, 26, 50, 76, 102, 130]
            for ci, (a, b) in enumerate(zip(bounds[:-1], bounds[1:])):
                q = nc.sync if ci % 2 == 0 else nc.scalar
                q.dma_start(xp[:, a:b, :], xv[:, a:b, :])

            # PE p-state warmup: dummy matmuls on memset tiles while the
            # input DMAs land, so the real loop starts at full clock.
            warm = sb.tile([128, 512], F16, tag="warm")
            nc.vector.memset(warm[:], 0.0)
            for wi in range(3):
                wps = cpsum.tile([128, 4, W], F32, tag="cp")
                nc.tensor.matmul(
                    wps[:], warm[:, :128],
                    warm.rearrange("k (r w) -> k r w", r=4),
                    start=True, stop=True,
                )

            for r0 in range(H // 4):
                ps = cpsum.tile([128, 4, W], F32, tag="cp")
                for t in range(9):
                    kh, kw = divmod(t, 3)
                    nc.tensor.matmul(
                        ps[:],
                        wst_sb[:, t, :],
                        xp[:, r0 * 4 + kh:r0 * 4 + kh + 4, kw:kw + W],
                        start=(t == 0),
                        stop=(t == 8),
                    )
                # batch two 4-row tiles into one 8-row store so the store
                # trigger count halves; alternate HWDGE queues
                if r0 % 2 == 0:
                    ot = outp.tile([128, 8, W], F16, tag="ot", name="ot")
                    nc.vector.tensor_copy(ot[:, 0:4], ps[:])
                else:
                    nc.scalar.activation(
                        ot[:, 4:8], ps[:], mybir.ActivationFunctionType.Copy
                    )
                    yq = nc.sync if (r0 // 2) % 2 == 0 else nc.scalar
                    yq.dma_start(yv[:, (r0 - 1) * 4:(r0 + 1) * 4, :], ot[:])
    nc.compile()
    return nc


def _get_programs():
    if "mlp" not in _cache:
        _cache["mlp"] = _build_mlp()
    if "conv" not in _cache:
        _cache["conv"] = _build_conv()
    return _cache["mlp"], _cache["conv"]


def kernel(x, cond, W1, b1, W2, b2, _trace=False):
    x = np.ascontiguousarray(np.asarray(x, dtype=np.float32))
    cond = np.asarray(cond, dtype=np.float32)
    W1 = np.asarray(W1, dtype=np.float32)
    b1 = np.asarray(b1, dtype=np.float32)
    W2 = np.asarray(W2, dtype=np.float32)
    b2 = np.asarray(b2, dtype=np.float32)

    nc_mlp, nc_conv = _get_programs()
    core_ids = list(range(NCORES))

    # host-side layout prep: every SBUF destination gets one contiguous
    # per-partition read
    condTS = np.ascontiguousarray(
        cond.T.reshape(2, 128, B).transpose(1, 0, 2)
    ).astype(np.float16)
    # [128, 36, 2, 128]: chunk-major, contiguous per partition per chunk-run
    W1TS = np.ascontiguousarray(
        W1.reshape(HCH, 128, 2, 128).transpose(3, 0, 2, 1)
    ).astype(np.float16)
    b1S = np.ascontiguousarray(b1.reshape(HCH, 128).T)
    # [8, 128, 36, 1152]: per-core pre-transposed W2 shard, fp8 e3m4 * 64
    W2TS = np.ascontiguousarray(
        (W2SCALE * W2).T.reshape(HCH, 128, NCORES, PSH).transpose(2, 1, 0, 3)
    ).astype(ml_dtypes.float8_e3m4)

    in_maps_a = [
        {"condT": condTS, "W1T": W1TS, "b1": b1S, "W2T": W2TS[i]}
        for i in core_ids
    ]
    res_a = run_bass_kernel_spmd(nc_mlp, in_maps_a, core_ids, trace=_trace)

    w = np.concatenate(
        [res_a.results[i]["wsh"].astype(np.float32) for i in core_ids], axis=1
    )
    w = w + b2[None, :]                      # [B, 9216]
    wr = w.reshape(B, COUT, CIN, 9)

    xpad = np.zeros((B, CIN, HP, WP), dtype=np.float16)
    xpad[:, :, 1:H + 1, 1:W + 1] = x

    in_maps_b = []
    for i in core_ids:
        blk = np.zeros((9, 128, 128), dtype=np.float16)
        for s in range(BSH):
            # [t, cin, cout] block for sample 4i+s on the diagonal
            blk[:, s * CIN:(s + 1) * CIN, s * COUT:(s + 1) * COUT] = (
                wr[i * BSH + s].transpose(2, 1, 0)
            )
        blkT = np.ascontiguousarray(blk.transpose(1, 0, 2))  # [128, 9, 128]
        in_maps_b.append({"xs": xpad[i * BSH:(i + 1) * BSH], "wst": blkT})
    res_b = run_bass_kernel_spmd(nc_conv, in_maps_b, core_ids, trace=_trace)

    out = np.concatenate(
        [res_b.results[i]["ys"].astype(np.float32) for i in core_ids], axis=0
    )
    if _trace:
        return out, (res_a, res_b)
    return out
